# revision 41
# baseline (speedup 1.0000x reference)
"""Trainium2 Bass kernel for a 2-layer 2-relation heterogeneous GCN with mean-pool head.

Sharding: destination nodes (and their incident edges) are sharded across 8
NeuronCores; the full feature table lives in each core's DRAM (layer-0 table is
the input x, the layer-1 table is assembled with an on-device AllGather).  The
small [128,128] weights are replicated.  Mean-pool partial sums are computed
per-core and summed on the host (the unshard step).

Per (output-tile, relation) on device:
  - dma_gather (4 SWDGE queues, int16 indices over two 25000-row table halves)
    pulls the source rows of all incident edges into SBUF, 128 edges/chunk.
  - DVE builds a norm-weighted selection matrix SelT[e,k] = (dstloc_e==k)*w_e.
  - PE accumulates aggT[din, node] = sum_chunks Msg_chunk^T-style matmuls.
  - PE applies W[l,r]; relu/bias on ACT/DVE; layer-1 fuses h2@lin_w and the
    per-graph mean-pool segment matmul into the same pass.
"""

import ml_dtypes
import numpy as np

import concourse.bacc as bacc
import concourse.bass as bass
import concourse.mybir as mybir
import concourse.tile as tile
from concourse.bass_utils import run_bass_kernel_spmd

P = 128
NCORES = 8
EDGE_DT = "bf16"  # bf16 edge pipeline: halves gather bytes, DVE and PE work

# Full-size problem constants (from the reference setup).
FULL = dict(N=50000, E=800000, R=2, L=2, D=128, G=64, C=8)


def _ceil_div(a, b):
    return -(-a // b)


def _prep(x, W, b, lin_w, lin_b, edge_index, batch, sizes):
    """Host-side index/normalization prep.  Returns (meta, in_maps)."""
    N, R, L, D, G, C = (sizes[k] for k in ("N", "R", "L", "D", "G", "C"))
    NS = N // NCORES
    HALF = N // 2
    TILES = _ceil_div(NS, P)
    # Layer-1 gathers read the two AllGather output buffers directly.  AG
    # chunk 0 covers local rows [0, AG0), chunk 1 [AG0, NS); its table rows
    # are ordered core-major: u = c*NS + lr  ->  (h, c*rows_h + lr - lo_h).
    # Tile-aligned split of the local rows.  A later split shrinks the exposed
    # tail AllGather, but NCORES*AG0 must stay under the int16 gather-index
    # limit (32767), so 31 tiles is the max.
    AG0 = 31 * P
    AGR = (AG0, NS - AG0)  # rows per chunk per core

    ei = np.asarray(edge_index, dtype=np.int64)
    batch_np = np.asarray(batch, dtype=np.int64)
    x = np.ascontiguousarray(np.asarray(x, dtype=np.float32))
    W = np.ascontiguousarray(np.asarray(W, dtype=np.float32))
    b = np.asarray(b, dtype=np.float32)
    lin_w = np.ascontiguousarray(np.asarray(lin_w, dtype=np.float32))
    lin_b = np.asarray(lin_b, dtype=np.float32)

    # Per-relation edges with symmetric normalization.  Self loops are NOT
    # materialized as edges: their contribution isd^2 * x[own] is injected on
    # device via an identity matmul over the (contiguous) own rows, which
    # saves ~10% of the gather/sel/matmul chunk volume.
    per_rel = []
    isds = []
    for r in range(R):
        src = ei[r, 0]
        dst = ei[r, 1]
        deg = np.bincount(dst, minlength=N).astype(np.float32) + 1.0
        isd = (1.0 / np.sqrt(deg)).astype(np.float32)
        w_e = isd[src] * isd[dst]
        per_rel.append((src, dst, w_e))
        isds.append(isd)

    # --- Balanced node -> (core, tile, slot) assignment -------------------
    # The dst partition is a free choice (the mean-pool output is permutation
    # invariant), so greedily balance per-(relation, src-half) in-degree
    # across the 8*TILES destination tiles.  Chunk counts are the max over
    # cores of per-(tile,half) edge counts, and the random assignment sits
    # just above the 8-chunk boundary -- balancing recovers ~15% of the
    # padded gather/sel/matmul volume.
    NBINS = NCORES * TILES
    capv = np.full(NBINS, P, dtype=np.int64)
    capv[TILES - 1 :: TILES] = NS - (TILES - 1) * P
    catv = np.zeros(N * 2 * R, dtype=np.int64)
    for r in range(R):
        src, dst, _ = per_rel[r]
        cat = 2 * r + (src // HALF)
        catv += np.bincount(dst * (2 * R) + cat, minlength=N * 2 * R)
    catv = catv.reshape(N, 2 * R)

    def greedy(cats):
        ncat = cats.shape[1]
        order = np.argsort(-cats.sum(1), kind="stable")
        loads = np.zeros((NBINS, ncat), dtype=np.float64)
        used = np.zeros(NBINS, dtype=np.int64)
        cdst = np.zeros(N, dtype=np.int64)
        slotc = np.zeros(N, dtype=np.int64)
        for u in order:
            lv = loads + cats[u]
            cand = (
                np.ceil(lv / P).sum(axis=1) * 4096.0
                + lv.max(axis=1)
                + np.where(used >= capv, 1e12, 0.0)
            )
            bn = int(np.argmin(cand))
            loads[bn] = lv[bn]
            cdst[u] = bn // TILES
            slotc[u] = (bn % TILES) * P + used[bn]
            used[bn] += 1
        return cdst, slotc

    cdst, slotc = greedy(catv)

    nodemap = np.full((NCORES, TILES * P), -1, dtype=np.int64)
    nodemap[cdst, slotc] = np.arange(N, dtype=np.int64)

    # Group edges by (core, tile, half); compute per-(l,r,t,h) chunk counts as
    # the max over cores so every core shares one program structure.  The half
    # split (and local index within the half table) differs per layer: layer 0
    # gathers from the two halves of x, layer 1 from the two AG buffers.
    grouped = [[[None] * NCORES for _ in range(R)] for _ in range(2)]
    cnts = np.zeros((2, R, NCORES, TILES, 2), dtype=np.int64)
    for r in range(R):
        s_all, d_all, w_all = per_rel[r]
        core = cdst[d_all]
        lr_all = slotc[s_all]  # layer-1 table local row of the src node
        h_l = [s_all // HALF, (lr_all >= AG0).astype(np.int64)]
        sloc_l = [
            s_all % HALF,
            np.where(
                lr_all < AG0,
                cdst[s_all] * AGR[0] + lr_all,
                cdst[s_all] * AGR[1] + (lr_all - AG0),
            ),
        ]
        for c in range(NCORES):
            m = core == c
            d = slotc[d_all[m]]
            w = w_all[m]
            t = d // P
            for l in range(2):
                h = h_l[l][m]
                s = sloc_l[l][m]
                key = (t * 2 + h).astype(np.int64)
                order2 = np.argsort(key, kind="stable")
                cnt = np.bincount(key[order2], minlength=TILES * 2).reshape(TILES, 2)
                cnts[l, r, c] = cnt
                grouped[l][r][c] = (s[order2], d[order2], w[order2], cnt)

    # chunks per (l, r, t, h): uniform across cores
    nch = np.maximum(_ceil_div(cnts.max(axis=2), P), 1)  # [2, R, TILES, 2]
    F_rt = nch.sum(axis=3)  # [2, R, TILES] chunks per (l, r, t)
    TOTF = int(F_rt.sum())

    # free-dim offsets per (l, r, t) into the concatenated arrays
    foff = np.zeros((2, R, TILES), dtype=np.int64)
    acc = 0
    for l in range(2):
        for r in range(R):
            for t in range(TILES):
                foff[l, r, t] = acc
                acc += int(F_rt[l, r, t])

    # Per-core packed arrays.
    in_maps = []
    edt_np = ml_dtypes.bfloat16 if EDGE_DT == "bf16" else np.float32
    b_sum = b.sum(axis=1)  # [L, D]
    counts = np.bincount(batch_np, minlength=G).astype(np.float32)
    icnt = (1.0 / np.maximum(counts, 1.0)).astype(np.float32)[:, None]  # [G,1]
    iota = np.tile(np.arange(P, dtype=np.float32)[None, :], (P, 1))  # [P,P] replicated
    ident = np.eye(P, dtype=edt_np)

    for c in range(NCORES):
        idx16 = np.zeros((P, TOTF * 8), dtype=np.int16)
        dlw = np.zeros((P, 2, TOTF), dtype=np.float32)
        # per-partition isd^2 for the self-loop identity matmuls: [P, R*TILES]
        isd2 = np.zeros((P, R * TILES), dtype=np.float32)
        for r in range(R):
            v = np.zeros(TILES * P, dtype=np.float32)
            v[:NS] = isds[r][nodemap[c, :NS]] ** 2
            isd2[:, r * TILES : (r + 1) * TILES] = v.reshape(TILES, P).T
        for l in range(2):
            for r in range(R):
                s, d, w, cnt = grouped[l][r][c]
                # start of each (t,h) group within this core's sorted edges
                gstart = np.concatenate([[0], np.cumsum(cnt.ravel())])[:-1].reshape(
                    TILES, 2
                )
                for t in range(TILES):
                    fo = int(foff[l, r, t])
                    ch_off = 0
                    for h in range(2):
                        k = int(nch[l, r, t, h])
                        n_real = int(cnt[t, h])
                        g0 = int(gstart[t, h])
                        # pad indices are -1: the gather ucode skips trailing
                        # negative indices, so each core only pays descriptor
                        # cost for its real edges (msg rows for skipped slots
                        # are stale; their sel weights are 0, and the msg pool
                        # is zeroed once at startup so they are never NaN).
                        sl = np.zeros(k * P, dtype=np.int64)  # NEGPAD disabled
                        dl = np.zeros(k * P, dtype=np.int64)
                        wl = np.zeros(k * P, dtype=np.float32)
                        sl[:n_real] = s[g0 : g0 + n_real]
                        if n_real == 0:
                            sl[0] = 0  # all-negative gathers can hang: keep one
                        dl[:n_real] = d[g0 : g0 + n_real] % P
                        wl[:n_real] = w[g0 : g0 + n_real]
                        # idx16 wrapped: idx i -> [i%16, i//16], replicated x8
                        iw = sl.astype(np.int16).reshape(k * 8, 16).T  # [16, k*8]
                        col0 = (fo + ch_off) * 8
                        idx16[:, col0 : col0 + k * 8] = np.tile(iw, (8, 1))
                        # dloc/w: edge e=j*128+p -> [p, j]
                        dlw[:, 0, fo + ch_off : fo + ch_off + k] = (
                            dl.astype(np.float32).reshape(k, P).T
                        )
                        dlw[:, 1, fo + ch_off : fo + ch_off + k] = wl.reshape(k, P).T
                        ch_off += k

        bl = np.full(TILES * P, -1.0, dtype=np.float32)
        bl[:NS] = batch_np[nodemap[c, :NS]].astype(np.float32)
        bloc = bl.reshape(TILES, P).T.copy()  # [P, TILES]

        in_maps.append(
            {
                "x": x if EDGE_DT == "f32" else np.zeros((1, 1), np.float32),
                "xh": x.astype(ml_dtypes.bfloat16) if EDGE_DT == "bf16" else np.zeros((1, 1), np.float32),
                "xol0": np.ascontiguousarray(x[nodemap[c, :NS]]).astype(edt_np),
                "Wt": W.astype(edt_np),
                "idx16": idx16,
                "dlw": dlw.astype(edt_np),
                "bloc": bloc,
                "icnt": icnt,
                "iota": iota,
                "iotah": iota.astype(edt_np),
                "ident": ident,
                "isd2": isd2,
                "linw": lin_w.astype(edt_np),
                "b0row": np.tile(b_sum[0][None, :], (P, 1)).copy(),
                "b1col": b_sum[1][:, None].copy(),
            }
        )

    meta = dict(
        N=N,
        NS=NS,
        HALF=HALF,
        AG0=AG0,
        AGR=AGR,
        TILES=TILES,
        R=R,
        D=D,
        G=G,
        C=C,
        TOTF=TOTF,
        nch=nch,
        F_rt=F_rt,
        foff=foff,
        has_b=bool(np.abs(b).max() > 0.0),
        edge_dt=EDGE_DT,
        lin_b=lin_b,
    )
    return meta, in_maps


def _build(meta):
    N = meta["N"]
    NS = meta["NS"]
    HALF = meta["HALF"]
    AG0 = meta["AG0"]
    AGR = meta["AGR"]
    TILES = meta["TILES"]
    R = meta["R"]
    D = meta["D"]
    G = meta["G"]
    C = meta["C"]
    TOTF = meta["TOTF"]
    nch = meta["nch"]
    F_rt = meta["F_rt"]
    foff = meta["foff"]
    has_b = meta["has_b"]
    f32 = mybir.dt.float32
    bf16 = mybir.dt.bfloat16
    edt = f32 if meta["edge_dt"] == "f32" else bf16

    nc = bacc.Bacc(
        "TRN2",
        target_bir_lowering=False,
        debug=False,
        num_devices=NCORES,
        num_swdge_queues=4,
        dynamic_dma_scratch_size=49152,
    )
    x_shape = [N, D] if meta["edge_dt"] == "f32" else [1, 1]
    x_ap = nc.dram_tensor("x", x_shape, f32, kind="ExternalInput").ap()
    xh_shape = [N, D] if meta["edge_dt"] == "bf16" else [1, 1]
    xh_dt = bf16 if meta["edge_dt"] == "bf16" else f32
    xh_ap = nc.dram_tensor("xh", xh_shape, xh_dt, kind="ExternalInput").ap()
    xol0 = nc.dram_tensor("xol0", [NS, D], edt, kind="ExternalInput").ap()
    Wt = nc.dram_tensor("Wt", [2, R, D, D], edt, kind="ExternalInput").ap()
    idx16 = nc.dram_tensor("idx16", [P, TOTF * 8], mybir.dt.int16, kind="ExternalInput").ap()
    dlw = nc.dram_tensor("dlw", [P, 2, TOTF], edt, kind="ExternalInput").ap()
    bloc = nc.dram_tensor("bloc", [P, TILES], f32, kind="ExternalInput").ap()
    icnt = nc.dram_tensor("icnt", [G, 1], f32, kind="ExternalInput").ap()
    iota = nc.dram_tensor("iota", [P, P], f32, kind="ExternalInput").ap()
    iotah = nc.dram_tensor("iotah", [P, P], edt, kind="ExternalInput").ap()
    ident = nc.dram_tensor("ident", [P, P], edt, kind="ExternalInput").ap()
    isd2 = nc.dram_tensor("isd2", [P, R * TILES], f32, kind="ExternalInput").ap()
    linw = nc.dram_tensor("linw", [D, C], edt, kind="ExternalInput").ap()
    b0row = nc.dram_tensor("b0row", [P, D], f32, kind="ExternalInput").ap()
    b1col = nc.dram_tensor("b1col", [D, 1], f32, kind="ExternalInput").ap()
    out_part = nc.dram_tensor("out_part", [G, C], f32, kind="ExternalOutput").ap()

    with tile.TileContext(nc) as tc:
        with (
            tc.tile_pool(name="const", bufs=1) as constp,
            tc.tile_pool(name="dram", bufs=1, space="DRAM") as dramp,
            tc.tile_pool(name="seld", bufs=14) as seldp,
            tc.tile_pool(name="idxp", bufs=14) as idxp,
            tc.tile_pool(name="selp", bufs=10) as selp,
            tc.tile_pool(name="msgp", bufs=14) as msgp,
            tc.tile_pool(name="aggs", bufs=6) as aggsp,
            tc.tile_pool(name="hnp", bufs=6) as hnp,
            tc.tile_pool(name="zp", bufs=2) as zp,
            tc.tile_pool(name="pselp", bufs=2) as pselp,
            tc.tile_pool(name="psagg", bufs=3, space="PSUM") as psagg,
            tc.tile_pool(name="pshn", bufs=2, space="PSUM") as pshn,
            tc.tile_pool(name="psz", bufs=2, space="PSUM") as psz,
            tc.tile_pool(name="pspool", bufs=1, space="PSUM") as pspool,
        ):
            # constants
            w_s = [[constp.tile([D, D], edt, tag=f"w{l}{r}", name=f"w{l}{r}") for r in range(R)] for l in range(2)]
            for l in range(2):
                for r in range(R):
                    nc.sync.dma_start(out=w_s[l][r][:], in_=Wt[l, r])
            linw_s = constp.tile([D, C], edt, tag="linw")
            nc.sync.dma_start(out=linw_s[:], in_=linw[:])
            iota_s = constp.tile([P, P], f32, tag="iota")
            nc.sync.dma_start(out=iota_s[:], in_=iota[:])
            iotah_s = constp.tile([P, P], edt, tag="iotah")
            nc.sync.dma_start(out=iotah_s[:], in_=iotah[:])
            ident_s = constp.tile([P, P], edt, tag="ident")
            nc.sync.dma_start(out=ident_s[:], in_=ident[:])
            isd2_s = constp.tile([P, R * TILES], f32, tag="isd2")
            nc.sync.dma_start(out=isd2_s[:], in_=isd2[:])
            bloc_s = constp.tile([P, TILES], f32, tag="bloc")
            nc.sync.dma_start(out=bloc_s[:], in_=bloc[:])
            icnt_s = constp.tile([G, 1], f32, tag="icnt")
            nc.sync.dma_start(out=icnt_s[:], in_=icnt[:])
            b0_s = constp.tile([P, D], f32, tag="b0")
            nc.sync.dma_start(out=b0_s[:], in_=b0row[:])
            b1_s = constp.tile([D, 1], f32, tag="b1")
            nc.sync.dma_start(out=b1_s[:], in_=b1col[:])

            AGC = 2
            B0 = AG0 // P
            ag_bounds = [B0, TILES]
            ag_rows = [(0, AG0), (AG0, NS)]
            h1own_q = [
                dramp.tile([hi - lo, D], edt, name=f"h1own{q}")
                for q, (lo, hi) in enumerate(ag_rows)
            ]
            h1ag = [
                dramp.tile([NCORES * (hi - lo), D], edt, name=f"h1ag{q}")
                for q, (lo, hi) in enumerate(ag_rows)
            ]
            pool_ps = pspool.tile([G, C], f32)

            ag_done = [False] * AGC

            def emit_ag(q):
                nc.gpsimd.collective_compute(
                    "AllGather",
                    mybir.AluOpType.bypass,
                    replica_groups=[list(range(NCORES))],
                    ins=[h1own_q[q][:].opt()],
                    outs=[h1ag[q][:].opt()],
                )

            def do_layer(l, tables, xown_src):
                for t in range(TILES):
                    rows = min(P, NS - t * P)
                    # own rows for the self-loop term (shared by both relations)
                    xo = hnp.tile([P, D], edt, tag="xown")
                    nc.sync.dma_start(out=xo[:rows, :], in_=xown_src(t, rows))
                    agg_sb = []
                    for r in range(R):
                        F = int(F_rt[l, r, t])
                        fo = int(foff[l, r, t])
                        seld = seldp.tile([P, 2, F], edt, tag="seld")
                        nc.sync.dma_start(out=seld[:], in_=dlw[:, :, fo : fo + F])
                        idxt = idxp.tile([P, F * 8], mybir.dt.int16, tag="idx")
                        nc.sync.dma_start(out=idxt[:, :], in_=idx16[:, fo * 8 : (fo + F) * 8])
                        sel = selp.tile([P, F, P], edt, tag="sel")
                        nc.vector.tensor_tensor(
                            out=sel[:],
                            in0=seld[:, 0, :].unsqueeze(2).to_broadcast([P, F, P]),
                            in1=iotah_s[:, :].unsqueeze(1).to_broadcast([P, F, P]),
                            op=mybir.AluOpType.is_equal,
                        )
                        nc.vector.tensor_tensor(
                            out=sel[:],
                            in0=sel[:],
                            in1=seld[:, 1, :].unsqueeze(2).to_broadcast([P, F, P]),
                            op=mybir.AluOpType.mult,
                        )
                        ks = [int(nch[l, r, t, 0]), int(nch[l, r, t, 1])]
                        msgs = []
                        for h in range(2):
                            k = ks[h]
                            msg_h = msgp.tile([P, k, D], edt, tag="msg")
                            nc.gpsimd.dma_gather(
                                out_ap=msg_h[:],
                                in_ap=tables[h],
                                idxs_ap=idxt[:, (0 if h == 0 else ks[0]) * 8 : (ks[0] + (ks[1] if h else 0)) * 8],
                                num_idxs=k * P,
                                num_idxs_reg=k * P,
                                elem_size=D,
                                queue_num=(2 * r + h + t) % 4,
                                single_packet=False,
                            )
                            msgs.append(msg_h)
                        # self-loop contribution: isd_r^2 * x[own] via identity matmul
                        xos = hnp.tile([P, D], edt, tag="xos")
                        nc.scalar.activation(
                            out=xos[:rows, :],
                            in_=xo[:rows, :],
                            func=mybir.ActivationFunctionType.Copy,
                            scale=isd2_s[:rows, r * TILES + t : r * TILES + t + 1],
                        )
                        agg_ps = psagg.tile([D, P], f32, tag="agg")
                        for h in range(2):
                            coff = 0 if h == 0 else ks[0]
                            for j in range(ks[h]):
                                nc.tensor.matmul(
                                    out=agg_ps[:],
                                    lhsT=msgs[h][:, j, :],
                                    rhs=sel[:, coff + j, :],
                                    start=(h == 0 and j == 0),
                                    stop=False,
                                )
                        nc.tensor.matmul(
                            out=agg_ps[:],
                            lhsT=xos[:rows, :],
                            rhs=ident_s[:rows, :],
                            start=False,
                            stop=True,
                        )
                        a_s = aggsp.tile([D, P], edt, tag="aggs")
                        nc.vector.tensor_copy(out=a_s[:], in_=agg_ps[:])
                        agg_sb.append(a_s)

                    if l == 0:
                        hn_ps = pshn.tile([P, D], f32, tag="hn")
                        for r in range(R):
                            nc.tensor.matmul(
                                out=hn_ps[:],
                                lhsT=agg_sb[r][:],
                                rhs=w_s[0][r][:],
                                start=(r == 0),
                                stop=(r == R - 1),
                            )
                        hn = hnp.tile([P, D], edt, tag="hnsb")
                        if has_b:
                            hb = hnp.tile([P, D], f32, tag="hbias")
                            nc.vector.tensor_tensor(
                                out=hb[:], in0=hn_ps[:], in1=b0_s[:],
                                op=mybir.AluOpType.add,
                            )
                            nc.scalar.activation(
                                out=hn[:], in_=hb[:], func=mybir.ActivationFunctionType.Relu
                            )
                        else:
                            nc.scalar.activation(
                                out=hn[:], in_=hn_ps[:], func=mybir.ActivationFunctionType.Relu
                            )
                        qi = next(i for i, b in enumerate(ag_bounds) if t < b)
                        q_lo = ag_rows[qi][0]
                        nc.sync.dma_start(
                            out=h1own_q[qi][t * P - q_lo : t * P - q_lo + rows, :],
                            in_=hn[:rows, :],
                        )
                        for _q, _b in enumerate(ag_bounds):
                            if t + 1 == _b + 2 and not ag_done[_q]:
                                emit_ag(_q)
                                ag_done[_q] = True
                    else:
                        h2_ps = pshn.tile([D, P], f32, tag="hn")
                        for r in range(R):
                            nc.tensor.matmul(
                                out=h2_ps[:],
                                lhsT=w_s[1][r][:],
                                rhs=agg_sb[r][:],
                                start=(r == 0),
                                stop=(r == R - 1),
                            )
                        h2t = hnp.tile([D, P], edt, tag="hnsb")
                        if has_b:
                            nc.scalar.activation(
                                out=h2t[:],
                                in_=h2_ps[:],
                                func=mybir.ActivationFunctionType.Copy,
                                bias=b1_s[:, :1],
                            )
                        else:
                            nc.vector.tensor_copy(out=h2t[:], in_=h2_ps[:])
                        z_ps = psz.tile([P, C], f32, tag="z")
                        nc.tensor.matmul(
                            out=z_ps[:], lhsT=h2t[:], rhs=linw_s[:], start=True, stop=True
                        )
                        z_s = zp.tile([P, C], f32, tag="zs")
                        nc.vector.tensor_copy(out=z_s[:], in_=z_ps[:])
                        psel = pselp.tile([P, G], f32, tag="psel")
                        nc.vector.tensor_tensor(
                            out=psel[:],
                            in0=bloc_s[:, t : t + 1].to_broadcast([P, G]),
                            in1=iota_s[:, :G],
                            op=mybir.AluOpType.is_equal,
                        )
                        nc.tensor.matmul(
                            out=pool_ps[:],
                            lhsT=psel[:],
                            rhs=z_s[:],
                            start=(t == 0),
                            stop=(t == TILES - 1),
                        )

            # zero all msg pool buffers once so slots skipped by negative pad
            # indices never read NaN garbage
            KMAX = int(nch.max())
            for i in range(14):
                mz = msgp.tile([P, KMAX, D], edt, tag="msg", name=f"msgz{i}")
                nc.vector.memset(mz[:], 0.0)

            l0tab = x_ap if meta["edge_dt"] == "f32" else xh_ap
            do_layer(
                0,
                (l0tab[0:HALF, :], l0tab[HALF:N, :]),
                lambda t, rows: xol0[t * P : t * P + rows, :],
            )
            for q in range(AGC):
                if not ag_done[q]:
                    emit_ag(q)
                    ag_done[q] = True

            def l1_xown(t, rows):
                qi = 0 if t < B0 else 1
                q_lo = ag_rows[qi][0]
                return h1own_q[qi][t * P - q_lo : t * P - q_lo + rows, :]

            do_layer(1, (h1ag[0][:], h1ag[1][:]), l1_xown)

            pool_s = zp.tile([G, C], f32, tag="pool")
            nc.vector.tensor_copy(out=pool_s[:], in_=pool_ps[:])
            nc.vector.tensor_scalar_mul(out=pool_s[:], in0=pool_s[:], scalar1=icnt_s[:, :1])
            nc.sync.dma_start(out=out_part[:], in_=pool_s[:])

    nc.compile()
    return nc


_CACHE = {}


def _run(x, W, b, lin_w, lin_b, edge_index, batch, sizes, trace=False):
    meta, in_maps = _prep(x, W, b, lin_w, lin_b, edge_index, batch, sizes)
    key = (sizes["N"], meta["TOTF"], tuple(meta["nch"].ravel().tolist()), meta["has_b"])
    nc = _CACHE.get(key)
    if nc is None:
        nc = _build(meta)
        _CACHE[key] = nc
    res = run_bass_kernel_spmd(
        nc, in_maps, core_ids=list(range(NCORES)), trace=trace
    )
    parts = [res.results[c]["out_part"] for c in range(NCORES)]
    out = np.sum(parts, axis=0) + np.asarray(lin_b, dtype=np.float32)[None, :]
    return out.astype(np.float32), res


def kernel(x, W, b, lin_w, lin_b, edge_index, batch):
    out, _ = _run(x, W, b, lin_w, lin_b, edge_index, batch, FULL)
    return out

